# revision 35
# baseline (speedup 1.0000x reference)
"""AttentionLayer Bass kernel for 8 trn2 NeuronCores.

Math (per example b):
    rinv[s]  = 1/max(||df[b,s,:]||, eps)
    le_n     = le / max(||le[:,l]||, eps)          (per column l)
    dots     = df @ le_n                            [S,3]
    atten    = dots * rinv[:,None] + 10000*(mask-1)[:,None]
    pooled   = max_l atten                          [S]
    normalized = softmax_s(pooled)
    feature_attention = normalized @ df             [H]

Sharding: data-parallel over batch B=64 -> 8 examples per core.

Layout strategy per example:
  - df loaded natural [128s, 8c, 1024h] (one 4MB DMA)
  - row sumsq via ACT Square+accum (5 chunks) + GPSIMD scalar_tensor_tensor
    (3 chunks)
  - df^T built with 64 PE transposes (fp32, 2cyc/row) -> PSUM -> ACT/DVE
    copies into dfT [128h, 8hc? -> [128, 8, 1024]] stripes
  - dots^T = le_n^T @ df^T via 16 accumulating float32r matmuls (1cyc/row)
  - dots^T -> natural via 8 tiny PE transposes; epilogue (rinv scale + mask
    bias) in natural layout on DVE; pooled = reduce_max
  - softmax without max-subtraction (|atten| <= 1 + nonpositive bias)
  - feature_attention via 16 accumulating float32r matmuls with exp as
    stationary [128,1] and natural df as moving operand
"""

import sys

sys.path.insert(0, "/opt/trn_rl_repo")

import numpy as np

import concourse.bass as bass
from concourse import mybir
from concourse.bass_types import AP
from concourse.bass_utils import run_bass_kernel_spmd
from concourse.masks import make_identity
from concourse.tile import TileContext

F32 = mybir.dt.float32
BF16 = mybir.dt.bfloat16
AF = mybir.ActivationFunctionType
ALU = mybir.AluOpType
AX = mybir.AxisListType

N_CORES = 8
B, S, H, L = 64, 1024, 1024, 3
B_LOC = B // N_CORES  # 8 examples per core
P = 128
NC = S // P  # 8 chunks (same count for S and H)
EPS = 1e-12

_CACHE = {}

# --- workaround: this walrus build allows at most ONE sync wait per
# instruction ("Too many sync wait commands").  Tile attaches several.
# Split extra waits onto standalone NoOps (same engine, just before the
# instruction) at BIR-serialization time.
_ORIG_TO_JSON_BYTES = bass.Bass.to_json_bytes


def _to_json_bytes_split_waits(self):
    import orjson

    j = orjson.loads(_ORIG_TO_JSON_BYTES(self))
    n = 0
    for fn in j["functions"]:
        for blk in fn["blocks"]:
            insts = blk["instructions"]
            if not any(
                len((i.get("sync_info") or {}).get("on_wait") or []) > 1
                for i in insts
            ):
                continue
            out = []
            for inst in insts:
                si = inst.get("sync_info")
                waits = (si or {}).get("on_wait") or []
                if len(waits) > 1:
                    for w in waits[:-1]:
                        n += 1
                        out.append(
                            {
                                "debug": inst.get("debug", 0),
                                "engine": inst["engine"],
                                "ins": [],
                                "outs": [],
                                "name": f"WSPLIT-{n}",
                                "opcode": "NoOp",
                                "sync_info": {"on_update": [], "on_wait": [w]},
                            }
                        )
                    si["on_wait"] = [waits[-1]]
                out.append(inst)
            blk["instructions"] = out
    return orjson.dumps(j)


bass.Bass.to_json_bytes = _to_json_bytes_split_waits


def _bcast3(ap2: AP, n: int) -> AP:
    """[P, C] AP -> [P, C, n] with a 0-step broadcast innermost dim."""
    return AP(tensor=ap2.tensor, offset=ap2.offset, ap=[*ap2.ap, [0, n]])


def _build():
    nc = bass.Bass("TRN2", target_bir_lowering=False, debug=False)

    df = nc.dram_tensor("deep_features", [B_LOC, S, H], F32, kind="ExternalInput").ap()
    le = nc.dram_tensor("label_embeddings", [H, L], F32, kind="ExternalInput").ap()
    mk = nc.dram_tensor("attention_mask", [B_LOC, S], F32, kind="ExternalInput").ap()
    fa = nc.dram_tensor("feature_attention", [B_LOC, H], F32, kind="ExternalOutput").ap()
    at = nc.dram_tensor("atten", [B_LOC, S, L], F32, kind="ExternalOutput").ap()
    nm = nc.dram_tensor("normalized", [B_LOC, 1, S], F32, kind="ExternalOutput").ap()

    # DRAM views
    # s index mapping: s = p*NC + c  (contiguous 32KB per partition in DRAM)
    df_v = df.rearrange("b (p c) h -> b p c h", c=NC)       # [B,128,8,1024]
    at_v = at.rearrange("b (p c) l -> b p c l", c=NC)       # [B,128,8,3]
    mk_v = mk.rearrange("b (p c) -> b p c", c=NC)           # [B,128,8]
    nm_v = nm.rearrange("b one (p c) -> b p (one c)", c=NC) # [B,128,8]
    le_v = le.rearrange("(c p) l -> p c l", p=P)            # [128,8,3]

    with TileContext(nc) as tc:
        with (
            tc.tile_pool(name="consts", bufs=1) as consts,
            tc.tile_pool(name="dfp", bufs=4) as dfp,
            tc.tile_pool(name="dftp", bufs=3) as dftp,
            tc.tile_pool(name="scr", bufs=2) as scr,
            tc.tile_pool(name="small", bufs=4) as small,
            tc.tile_pool(name="outs", bufs=3) as outs,
            tc.tile_pool(name="ps_tr", bufs=3, space="PSUM") as ps_tr,
            tc.tile_pool(name="ps_dots", bufs=1, space="PSUM") as ps_dots,
            tc.tile_pool(name="ps_fa", bufs=1, space="PSUM") as ps_fa,
        ):
            # ---- constants ----
            ident = consts.tile([P, P], F32)
            make_identity(nc, ident)
            identb = consts.tile([P, P], BF16)
            make_identity(nc, identb)
            ones_big = consts.tile([P, P], F32)
            nc.vector.memset(ones_big, 1.0)

            # ---- normalize label embeddings once ----
            le_sb = consts.tile([P, NC, L], F32)
            nc.sync.dma_start(out=le_sb, in_=le_v)
            le_sq = consts.tile([P, NC, L], F32)
            nc.scalar.activation(le_sq, le_sb, AF.Square)
            # per-partition partial sums over c: view [p, l, c], reduce X
            le_sq_plc = AP(
                tensor=le_sq.tensor,
                offset=le_sq.offset,
                ap=[le_sq.ap[0], [1, L], [L, NC]],
            )
            le_part = consts.tile([P, L], F32)
            nc.vector.tensor_reduce(le_part, le_sq_plc, axis=AX.X, op=ALU.add)
            lep_tile = ps_fa.tile([P, L], F32, tag="fa")
            lep_ps = lep_tile[:, :]
            # all-ones stationary: every out partition gets the col sum
            nc.tensor.matmul(lep_ps, ones_big, le_part)  # [128,L] col sums
            # 1/max(sqrt(x), eps) == exp(-0.5*ln(max(x, eps^2))); stays in
            # the natural_log_exp ACT table set (no per-example set switch)
            le_lns = consts.tile([P, L], F32)
            nc.vector.tensor_scalar_max(le_lns, lep_ps, EPS * EPS)
            nc.scalar.activation(le_lns, le_lns, AF.Ln)
            le_rinv_bc = consts.tile([P, L], F32)
            nc.scalar.activation(le_rinv_bc, le_lns, AF.Exp, scale=-0.5)
            le_n = consts.tile([P, NC, L], F32)
            for c in range(NC):
                nc.vector.tensor_tensor(
                    out=le_n[:, c, :], in0=le_sb[:, c, :], in1=le_rinv_bc, op=ALU.mult
                )
            le_nb = consts.tile([P, NC, L], BF16)
            nc.vector.tensor_copy(le_nb, le_n)

            # ---- per-example pipeline (2-deep software pipeline) ----
            # PE emission order per step: [transposes+dots of b] then
            # [minis of b-1] then [wsum of b-2], so PE never waits on the
            # serial softmax chain of the current example.
            st = {}

            def phase_load(b):
                s = {}
                df_t = dfp.tile([P, NC, H], BF16, tag="df")
                nc.gpsimd.dma_start(out=df_t[:, 0:4, :], in_=df_v[b][:, 0:4, :])
                nc.gpsimd.dma_start(out=df_t[:, 4:8, :], in_=df_v[b][:, 4:8, :])
                s["df_t"] = df_t

                ss = small.tile([P, NC], F32, tag="ss")
                for c in range(NC):
                    sq = scr.tile([P, H], BF16, tag="sq")
                    if c < 4:
                        nc.scalar.activation(
                            sq, df_t[:, c, :], AF.Square, accum_out=ss[:, c : c + 1]
                        )
                    else:
                        nc.vector.scalar_tensor_tensor(
                            out=sq,
                            in0=df_t[:, c, :],
                            scalar=1.0,
                            in1=df_t[:, c, :],
                            op0=ALU.mult,
                            op1=ALU.mult,
                            accum_out=ss[:, c : c + 1],
                        )
                # rinv = 1/max(sqrt(ss), eps) = exp(-0.5*ln(max(ss, eps^2)))
                rt = small.tile([P, NC], F32, tag="rt")
                nc.vector.tensor_scalar_max(rt, ss, EPS * EPS)
                nc.scalar.activation(rt, rt, AF.Ln)
                rinv = small.tile([P, NC], F32, tag="rinv")
                nc.scalar.activation(rinv, rt, AF.Exp, scale=-0.5)
                s["rinv"] = rinv

                mask_nat = small.tile([P, NC], F32, tag="mask")
                nc.sync.dma_start(out=mask_nat, in_=mk_v[b])
                bias_nat = small.tile([P, NC], F32, tag="bias")
                nc.scalar.activation(
                    bias_nat, mask_nat, AF.Copy, bias=-10000.0, scale=10000.0
                )
                s["bias_nat"] = bias_nat
                return s

            def phase_transpose_dots(b, s):
                df_t = s["df_t"]
                dfT = dftp.tile([P, NC, S], BF16, tag="dfT")
                dots0 = ps_dots.tile([L, 512], F32, tag="dots0")
                dots1 = ps_dots.tile([L, 512], F32, tag="dots1")

                def dots_mm(hc):
                    lhsT = le_nb[:, hc, :]
                    nc.tensor.matmul(
                        dots0,
                        lhsT,
                        dfT[:, hc, 0:512],
                        start=(hc == 0),
                        stop=(hc == NC - 1),
                        skip_group_check=True,
                    )
                    nc.tensor.matmul(
                        dots1,
                        lhsT,
                        dfT[:, hc, 512:1024],
                        start=(hc == 0),
                        stop=(hc == NC - 1),
                        skip_group_check=True,
                    )

                # dots for hc are emitted one step behind the transposes so
                # the PSUM->SBUF copy of stripe hc hides under the hc+1
                # transpose burst (PE never waits on the copy)
                for hc in range(NC):
                    tr = ps_tr.tile([P, S], BF16, tag="tr")  # 2KB = 1 bank
                    for sc in range(NC):
                        nc.tensor.transpose(
                            tr[:, sc * P : (sc + 1) * P],
                            df_t[:, sc, hc * P : (hc + 1) * P],
                            identb,
                        )
                    dst = dfT[:, hc, :]
                    if hc % 2 == 0:
                        nc.scalar.copy(dst, tr)
                    else:
                        nc.vector.tensor_copy(dst, tr)
                    if hc > 0:
                        dots_mm(hc - 1)
                dots_mm(NC - 1)

                dotsT_sb = small.tile([L, S], F32, tag="dotsT")
                # scatter copy so dotsT_sb column k holds s=k (undo the
                # block ordering of the transposed stripes)
                dT_v = dotsT_sb.rearrange("l (j c) -> l c j", c=NC)
                nc.vector.tensor_copy(
                    dT_v[:, 0:4, :], dots0.rearrange("l (c j) -> l c j", c=4)
                )
                nc.vector.tensor_copy(
                    dT_v[:, 4:8, :], dots1.rearrange("l (c j) -> l c j", c=4)
                )
                s["dotsT_sb"] = dotsT_sb
            def phase_softmax(b, s):
                dotsT_sb = s["dotsT_sb"]
                # dots^T rows -> natural (p c) layout via partition-remap DMAs
                dnat_sb = small.tile([P, L, NC], F32, tag="dnat")
                for l in range(L):
                    nc.sync.dma_start(
                        out=dnat_sb[:, l, :], in_=dotsT_sb[l : l + 1, :]
                    )
                tmp3 = outs.tile([P, NC, L], F32, tag="tmp3")
                atnat = outs.tile([P, NC, L], F32, tag="atnat")
                for l in range(L):
                    nc.vector.tensor_tensor(
                        out=tmp3[:, :, l],
                        in0=dnat_sb[:, l, :],
                        in1=s["rinv"],
                        op=ALU.mult,
                    )
                    nc.vector.tensor_tensor(
                        out=atnat[:, :, l],
                        in0=tmp3[:, :, l],
                        in1=s["bias_nat"],
                        op=ALU.add,
                    )
                nc.scalar.dma_start(out=at_v[b], in_=atnat)
                pooled = small.tile([P, NC], F32, tag="pooled")
                nc.vector.tensor_reduce(pooled, atnat, axis=AX.X, op=ALU.max)

                exp_nat = small.tile([P, NC], F32, tag="exp")
                esum_col = small.tile([P, 1], F32, tag="esumc")
                nc.scalar.activation(exp_nat, pooled, AF.Exp, accum_out=esum_col)
                esum_tile = ps_dots.tile([P, 1], F32, tag="esum")
                esum_ps = esum_tile[:, :]
                nc.tensor.matmul(esum_ps, ones_big, esum_col)
                rz128 = small.tile([P, 1], F32, tag="rz128")
                nc.vector.reciprocal(rz128, esum_ps)
                s["rz128"] = rz128

                norm_nat = outs.tile([P, NC], F32, tag="norm")
                nc.vector.tensor_scalar_mul(norm_nat, exp_nat, rz128)
                nc.scalar.dma_start(out=nm_v[b], in_=norm_nat)

                exp_bf = small.tile([P, NC], BF16, tag="expbf")
                nc.vector.tensor_copy(exp_bf, exp_nat)
                s["exp_bf"] = exp_bf

            def phase_wsum(b, s):
                df_t, exp_bf, rz128 = s["df_t"], s["exp_bf"], s["rz128"]
                fa_ps = ps_fa.tile([1, H], F32, tag="fa")
                for c in range(NC):
                    lhsT = exp_bf[:, c : c + 1]
                    nc.tensor.matmul(
                        fa_ps[:, 0:512],
                        lhsT,
                        df_t[:, c, 0:512],
                        start=(c == 0),
                        stop=(c == NC - 1),
                        skip_group_check=True,
                    )
                    nc.tensor.matmul(
                        fa_ps[:, 512:1024],
                        lhsT,
                        df_t[:, c, 512:1024],
                        start=(c == 0),
                        stop=(c == NC - 1),
                        skip_group_check=True,
                    )
                fa_sb = outs.tile([1, H], F32, tag="fa_sb")
                nc.vector.tensor_scalar_mul(
                    fa_sb[:, 0:512], fa_ps[:, 0:512], rz128[0:1, :]
                )
                nc.vector.tensor_scalar_mul(
                    fa_sb[:, 512:1024], fa_ps[:, 512:1024], rz128[0:1, :]
                )
                nc.scalar.dma_start(out=fa[b : b + 1, :], in_=fa_sb)

            for b in range(B_LOC + 2):
                if b < B_LOC:
                    st[b] = phase_load(b)
                    phase_transpose_dots(b, st[b])
                if 1 <= b <= B_LOC:
                    phase_softmax(b - 1, st[b - 1])
                if b >= 2:
                    phase_wsum(b - 2, st[b - 2])
                    del st[b - 2]

    return nc


def get_nc():
    if "nc" not in _CACHE:
        _CACHE["nc"] = _build()
    return _CACHE["nc"]


def kernel(deep_features, label_embeddings, attention_mask, trace=False, **kw):
    deep_features = np.asarray(deep_features, dtype=np.float32)
    label_embeddings = np.asarray(label_embeddings, dtype=np.float32)
    attention_mask = np.asarray(attention_mask, dtype=np.float32)

    nc = get_nc()
    in_maps = []
    for i in range(N_CORES):
        sl = slice(i * B_LOC, (i + 1) * B_LOC)
        in_maps.append(
            {
                "deep_features": np.ascontiguousarray(deep_features[sl]),
                "label_embeddings": label_embeddings,
                "attention_mask": np.ascontiguousarray(attention_mask[sl]),
            }
        )
    res = run_bass_kernel_spmd(
        nc, in_maps, core_ids=list(range(N_CORES)), trace=trace, **kw
    )
    fa = np.concatenate([r["feature_attention"] for r in res.results], axis=0)
    at = np.concatenate([r["atten"] for r in res.results], axis=0)
    nm = np.concatenate([r["normalized"] for r in res.results], axis=0)
    if trace:
        kernel.last_results = res
    return fa, at, nm


# revision 36
# speedup vs baseline: 1.0125x; 1.0125x over previous
"""AttentionLayer Bass kernel for 8 trn2 NeuronCores.

Math (per example b):
    rinv[s]  = 1/max(||df[b,s,:]||, eps)
    le_n     = le / max(||le[:,l]||, eps)          (per column l)
    dots     = df @ le_n                            [S,3]
    atten    = dots * rinv[:,None] + 10000*(mask-1)[:,None]
    pooled   = max_l atten                          [S]
    normalized = softmax_s(pooled)
    feature_attention = normalized @ df             [H]

Sharding: data-parallel over batch B=64 -> 8 examples per core.

Layout strategy per example (s index mapping: s = p*8 + c):
  - df loaded as bf16 [128p, 8c, 1024h] via two SWDGE cast-DMAs (fp32->bf16)
  - row sumsq: ACT Square+accum (4 chunks) + DVE scalar_tensor_tensor (4)
  - df^T built with 64 PE transpose-mode matmuls (bf16) -> PSUM [128,1024]
    (1 bank) -> ACT/DVE copies into dfT [128h, 8hc, 1024s] stripes;
    dots matmuls lag one stripe behind so copies hide under transposes
  - dots^T = le_n^T @ df^T via 16 accumulating bf16 matmuls; PSUM->SBUF
    scatter-copy makes dotsT_sb column k hold s=k
  - dots^T rows -> natural [128,8] layout via 3 partition-remap DMAs
  - epilogue (rinv scale + mask bias) on DVE; pooled = reduce_max
  - softmax without max-subtraction (|atten| <= 1 + nonpositive bias);
    rinv and 1/Z via exp/ln only (single ACT table set, no reloads)
  - feature_attention via 16 accumulating bf16 matmuls (exp stationary)
  - 2-deep cross-example software pipeline: PE runs example b transposes,
    then b-1 dots-to-natural, then b-2 weighted sum
"""

import sys

sys.path.insert(0, "/opt/trn_rl_repo")

import numpy as np

import concourse.bass as bass
from concourse import mybir
from concourse.bass_types import AP
from concourse.bass_utils import run_bass_kernel_spmd
from concourse.masks import make_identity
from concourse.tile import TileContext

F32 = mybir.dt.float32
BF16 = mybir.dt.bfloat16
AF = mybir.ActivationFunctionType
ALU = mybir.AluOpType
AX = mybir.AxisListType

N_CORES = 8
B, S, H, L = 64, 1024, 1024, 3
B_LOC = B // N_CORES  # 8 examples per core
P = 128
NC = S // P  # 8 chunks (same count for S and H)
EPS = 1e-12

_CACHE = {}

# --- workaround: this walrus build allows at most ONE sync wait per
# instruction ("Too many sync wait commands").  Tile attaches several.
# Split extra waits onto standalone NoOps (same engine, just before the
# instruction) at BIR-serialization time.
_ORIG_TO_JSON_BYTES = bass.Bass.to_json_bytes


def _to_json_bytes_split_waits(self):
    import orjson

    j = orjson.loads(_ORIG_TO_JSON_BYTES(self))
    n = 0
    for fn in j["functions"]:
        for blk in fn["blocks"]:
            insts = blk["instructions"]
            if not any(
                len((i.get("sync_info") or {}).get("on_wait") or []) > 1
                for i in insts
            ):
                continue
            out = []
            for inst in insts:
                si = inst.get("sync_info")
                waits = (si or {}).get("on_wait") or []
                if len(waits) > 1:
                    for w in waits[:-1]:
                        n += 1
                        out.append(
                            {
                                "debug": inst.get("debug", 0),
                                "engine": inst["engine"],
                                "ins": [],
                                "outs": [],
                                "name": f"WSPLIT-{n}",
                                "opcode": "NoOp",
                                "sync_info": {"on_update": [], "on_wait": [w]},
                            }
                        )
                    si["on_wait"] = [waits[-1]]
                out.append(inst)
            blk["instructions"] = out
    return orjson.dumps(j)


bass.Bass.to_json_bytes = _to_json_bytes_split_waits


def _bcast3(ap2: AP, n: int) -> AP:
    """[P, C] AP -> [P, C, n] with a 0-step broadcast innermost dim."""
    return AP(tensor=ap2.tensor, offset=ap2.offset, ap=[*ap2.ap, [0, n]])


def _build():
    nc = bass.Bass("TRN2", target_bir_lowering=False, debug=False)

    df = nc.dram_tensor("deep_features", [B_LOC, S, H], F32, kind="ExternalInput").ap()
    le = nc.dram_tensor("label_embeddings", [H, L], F32, kind="ExternalInput").ap()
    mk = nc.dram_tensor("attention_mask", [B_LOC, S], F32, kind="ExternalInput").ap()
    fa = nc.dram_tensor("feature_attention", [B_LOC, H], F32, kind="ExternalOutput").ap()
    at = nc.dram_tensor("atten", [B_LOC, S, L], F32, kind="ExternalOutput").ap()
    nm = nc.dram_tensor("normalized", [B_LOC, 1, S], F32, kind="ExternalOutput").ap()

    # DRAM views
    # s index mapping: s = p*NC + c  (contiguous 32KB per partition in DRAM)
    df_v = df.rearrange("b (p c) h -> b p c h", c=NC)       # [B,128,8,1024]
    at_v = at.rearrange("b (p c) l -> b p c l", c=NC)       # [B,128,8,3]
    mk_v = mk.rearrange("b (p c) -> b p c", c=NC)           # [B,128,8]
    nm_v = nm.rearrange("b one (p c) -> b p (one c)", c=NC) # [B,128,8]
    le_v = le.rearrange("(c p) l -> p c l", p=P)            # [128,8,3]

    with TileContext(nc) as tc:
        with (
            tc.tile_pool(name="consts", bufs=1) as consts,
            tc.tile_pool(name="dfp", bufs=4) as dfp,
            tc.tile_pool(name="dftp", bufs=2) as dftp,
            tc.tile_pool(name="scr", bufs=2) as scr,
            tc.tile_pool(name="small", bufs=3) as small,
            tc.tile_pool(name="outs", bufs=3) as outs,
            tc.tile_pool(name="ps_tr", bufs=3, space="PSUM") as ps_tr,
            tc.tile_pool(name="ps_dots", bufs=1, space="PSUM") as ps_dots,
            tc.tile_pool(name="ps_fa", bufs=1, space="PSUM") as ps_fa,
        ):
            # ---- constants ----
            ident = consts.tile([P, P], F32)
            make_identity(nc, ident)
            identb = consts.tile([P, P], BF16)
            make_identity(nc, identb)
            ones_big = consts.tile([P, P], F32)
            nc.vector.memset(ones_big, 1.0)

            # ---- normalize label embeddings once ----
            le_sb = consts.tile([P, NC, L], F32)
            nc.sync.dma_start(out=le_sb, in_=le_v)
            le_sq = consts.tile([P, NC, L], F32)
            nc.scalar.activation(le_sq, le_sb, AF.Square)
            # per-partition partial sums over c: view [p, l, c], reduce X
            le_sq_plc = AP(
                tensor=le_sq.tensor,
                offset=le_sq.offset,
                ap=[le_sq.ap[0], [1, L], [L, NC]],
            )
            le_part = consts.tile([P, L], F32)
            nc.vector.tensor_reduce(le_part, le_sq_plc, axis=AX.X, op=ALU.add)
            lep_tile = ps_fa.tile([P, L], F32, tag="fa")
            lep_ps = lep_tile[:, :]
            # all-ones stationary: every out partition gets the col sum
            nc.tensor.matmul(lep_ps, ones_big, le_part)  # [128,L] col sums
            # 1/max(sqrt(x), eps) == exp(-0.5*ln(max(x, eps^2))); stays in
            # the natural_log_exp ACT table set (no per-example set switch)
            le_lns = consts.tile([P, L], F32)
            nc.vector.tensor_scalar_max(le_lns, lep_ps, EPS * EPS)
            nc.scalar.activation(le_lns, le_lns, AF.Ln)
            le_rinv_bc = consts.tile([P, L], F32)
            nc.scalar.activation(le_rinv_bc, le_lns, AF.Exp, scale=-0.5)
            le_n = consts.tile([P, NC, L], F32)
            for c in range(NC):
                nc.vector.tensor_tensor(
                    out=le_n[:, c, :], in0=le_sb[:, c, :], in1=le_rinv_bc, op=ALU.mult
                )
            le_nb = consts.tile([P, NC, L], BF16)
            nc.vector.tensor_copy(le_nb, le_n)

            # ---- per-example pipeline (2-deep software pipeline) ----
            # PE emission order per step: [transposes+dots of b] then
            # [minis of b-1] then [wsum of b-2], so PE never waits on the
            # serial softmax chain of the current example.
            st = {}

            def phase_load(b):
                s = {}
                df_t = dfp.tile([P, NC, H], BF16, tag="df")
                nc.gpsimd.dma_start(out=df_t[:, 0:4, :], in_=df_v[b][:, 0:4, :])
                nc.gpsimd.dma_start(out=df_t[:, 4:8, :], in_=df_v[b][:, 4:8, :])
                s["df_t"] = df_t

                ss = small.tile([P, NC], F32, tag="ss")
                for c in range(NC):
                    sq = scr.tile([P, H], BF16, tag="sq")
                    if c < 4:
                        nc.scalar.activation(
                            sq, df_t[:, c, :], AF.Square, accum_out=ss[:, c : c + 1]
                        )
                    else:
                        nc.vector.scalar_tensor_tensor(
                            out=sq,
                            in0=df_t[:, c, :],
                            scalar=1.0,
                            in1=df_t[:, c, :],
                            op0=ALU.mult,
                            op1=ALU.mult,
                            accum_out=ss[:, c : c + 1],
                        )
                # rinv = 1/max(sqrt(ss), eps) = exp(-0.5*ln(max(ss, eps^2)))
                rt = small.tile([P, NC], F32, tag="rt")
                nc.vector.tensor_scalar_max(rt, ss, EPS * EPS)
                nc.scalar.activation(rt, rt, AF.Ln)
                rinv = small.tile([P, NC], F32, tag="rinv")
                nc.scalar.activation(rinv, rt, AF.Exp, scale=-0.5)
                s["rinv"] = rinv

                mask_nat = small.tile([P, NC], F32, tag="mask")
                nc.sync.dma_start(out=mask_nat, in_=mk_v[b])
                bias_nat = small.tile([P, NC], F32, tag="bias")
                nc.scalar.activation(
                    bias_nat, mask_nat, AF.Copy, bias=-10000.0, scale=10000.0
                )
                s["bias_nat"] = bias_nat
                return s

            def phase_transpose_dots(b, s):
                df_t = s["df_t"]
                dfT = dftp.tile([P, NC, S], BF16, tag="dfT")
                dots0 = ps_dots.tile([L, 512], F32, tag="dots0")
                dots1 = ps_dots.tile([L, 512], F32, tag="dots1")

                def dots_mm(hc):
                    lhsT = le_nb[:, hc, :]
                    nc.tensor.matmul(
                        dots0,
                        lhsT,
                        dfT[:, hc, 0:512],
                        start=(hc == 0),
                        stop=(hc == NC - 1),
                        skip_group_check=True,
                    )
                    nc.tensor.matmul(
                        dots1,
                        lhsT,
                        dfT[:, hc, 512:1024],
                        start=(hc == 0),
                        stop=(hc == NC - 1),
                        skip_group_check=True,
                    )

                # dots for hc are emitted one step behind the transposes so
                # the PSUM->SBUF copy of stripe hc hides under the hc+1
                # transpose burst (PE never waits on the copy)
                for hc in range(NC):
                    tr = ps_tr.tile([P, S], BF16, tag="tr")  # 2KB = 1 bank
                    for sc in range(NC):
                        nc.tensor.transpose(
                            tr[:, sc * P : (sc + 1) * P],
                            df_t[:, sc, hc * P : (hc + 1) * P],
                            identb,
                        )
                    dst = dfT[:, hc, :]
                    if hc % 2 == 0:
                        nc.scalar.copy(dst, tr)
                    else:
                        nc.vector.tensor_copy(dst, tr)
                    if hc > 0:
                        dots_mm(hc - 1)
                dots_mm(NC - 1)

                dotsT_sb = small.tile([L, S], F32, tag="dotsT")
                # scatter copy so dotsT_sb column k holds s=k (undo the
                # block ordering of the transposed stripes)
                dT_v = dotsT_sb.rearrange("l (j c) -> l c j", c=NC)
                nc.vector.tensor_copy(
                    dT_v[:, 0:4, :], dots0.rearrange("l (c j) -> l c j", c=4)
                )
                nc.vector.tensor_copy(
                    dT_v[:, 4:8, :], dots1.rearrange("l (c j) -> l c j", c=4)
                )
                s["dotsT_sb"] = dotsT_sb
            def phase_softmax(b, s):
                dotsT_sb = s["dotsT_sb"]
                # dots^T rows -> natural (p c) layout via partition-remap DMAs
                dnat_sb = small.tile([P, L, NC], F32, tag="dnat")
                for l in range(L):
                    nc.sync.dma_start(
                        out=dnat_sb[:, l, :], in_=dotsT_sb[l : l + 1, :]
                    )
                tmp3 = outs.tile([P, NC, L], F32, tag="tmp3")
                atnat = outs.tile([P, NC, L], F32, tag="atnat")
                for l in range(L):
                    nc.vector.tensor_tensor(
                        out=tmp3[:, :, l],
                        in0=dnat_sb[:, l, :],
                        in1=s["rinv"],
                        op=ALU.mult,
                    )
                    nc.vector.tensor_tensor(
                        out=atnat[:, :, l],
                        in0=tmp3[:, :, l],
                        in1=s["bias_nat"],
                        op=ALU.add,
                    )
                nc.scalar.dma_start(out=at_v[b], in_=atnat)
                pooled = small.tile([P, NC], F32, tag="pooled")
                nc.vector.tensor_reduce(pooled, atnat, axis=AX.X, op=ALU.max)

                exp_nat = small.tile([P, NC], F32, tag="exp")
                esum_col = small.tile([P, 1], F32, tag="esumc")
                nc.scalar.activation(exp_nat, pooled, AF.Exp, accum_out=esum_col)
                esum_tile = ps_dots.tile([P, 1], F32, tag="esum")
                esum_ps = esum_tile[:, :]
                nc.tensor.matmul(esum_ps, ones_big, esum_col)
                rz128 = small.tile([P, 1], F32, tag="rz128")
                nc.vector.reciprocal(rz128, esum_ps)
                s["rz128"] = rz128

                norm_nat = outs.tile([P, NC], F32, tag="norm")
                nc.vector.tensor_scalar_mul(norm_nat, exp_nat, rz128)
                nc.scalar.dma_start(out=nm_v[b], in_=norm_nat)

                exp_bf = small.tile([P, NC], BF16, tag="expbf")
                nc.vector.tensor_copy(exp_bf, exp_nat)
                s["exp_bf"] = exp_bf

            def phase_wsum(b, s):
                df_t, exp_bf, rz128 = s["df_t"], s["exp_bf"], s["rz128"]
                fa_ps = ps_fa.tile([1, H], F32, tag="fa")
                for c in range(NC):
                    lhsT = exp_bf[:, c : c + 1]
                    nc.tensor.matmul(
                        fa_ps[:, 0:512],
                        lhsT,
                        df_t[:, c, 0:512],
                        start=(c == 0),
                        stop=(c == NC - 1),
                        skip_group_check=True,
                    )
                    nc.tensor.matmul(
                        fa_ps[:, 512:1024],
                        lhsT,
                        df_t[:, c, 512:1024],
                        start=(c == 0),
                        stop=(c == NC - 1),
                        skip_group_check=True,
                    )
                fa_sb = outs.tile([1, H], F32, tag="fa_sb")
                nc.vector.tensor_scalar_mul(
                    fa_sb[:, 0:512], fa_ps[:, 0:512], rz128[0:1, :]
                )
                nc.vector.tensor_scalar_mul(
                    fa_sb[:, 512:1024], fa_ps[:, 512:1024], rz128[0:1, :]
                )
                nc.scalar.dma_start(out=fa[b : b + 1, :], in_=fa_sb)

            for b in range(B_LOC + 2):
                if b < B_LOC:
                    st[b] = phase_load(b)
                    phase_transpose_dots(b, st[b])
                if 1 <= b <= B_LOC:
                    phase_softmax(b - 1, st[b - 1])
                if b >= 2:
                    phase_wsum(b - 2, st[b - 2])
                    del st[b - 2]

    return nc


def get_nc():
    if "nc" not in _CACHE:
        _CACHE["nc"] = _build()
    return _CACHE["nc"]


def kernel(deep_features, label_embeddings, attention_mask, trace=False, **kw):
    deep_features = np.asarray(deep_features, dtype=np.float32)
    label_embeddings = np.asarray(label_embeddings, dtype=np.float32)
    attention_mask = np.asarray(attention_mask, dtype=np.float32)

    nc = get_nc()
    in_maps = []
    for i in range(N_CORES):
        sl = slice(i * B_LOC, (i + 1) * B_LOC)
        in_maps.append(
            {
                "deep_features": np.ascontiguousarray(deep_features[sl]),
                "label_embeddings": label_embeddings,
                "attention_mask": np.ascontiguousarray(attention_mask[sl]),
            }
        )
    res = run_bass_kernel_spmd(
        nc, in_maps, core_ids=list(range(N_CORES)), trace=trace, **kw
    )
    fa = np.concatenate([r["feature_attention"] for r in res.results], axis=0)
    at = np.concatenate([r["atten"] for r in res.results], axis=0)
    nm = np.concatenate([r["normalized"] for r in res.results], axis=0)
    if trace:
        kernel.last_results = res
    return fa, at, nm


# revision 37
# speedup vs baseline: 1.0539x; 1.0410x over previous
"""AttentionLayer Bass kernel for 8 trn2 NeuronCores.

Math (per example b):
    rinv[s]  = 1/max(||df[b,s,:]||, eps)
    le_n     = le / max(||le[:,l]||, eps)          (per column l)
    dots     = df @ le_n                            [S,3]
    atten    = dots * rinv[:,None] + 10000*(mask-1)[:,None]
    pooled   = max_l atten                          [S]
    normalized = softmax_s(pooled)
    feature_attention = normalized @ df             [H]

Sharding: data-parallel over batch B=64 -> 8 examples per core.

Layout strategy per example (s index mapping: s = p*8 + c):
  - df loaded as bf16 [128p, 8c, 1024h] via two SWDGE cast-DMAs (fp32->bf16)
  - row sumsq: ACT Square+accum (4 chunks) + DVE scalar_tensor_tensor (4)
  - df^T built with 64 PE transpose-mode matmuls (bf16) -> PSUM [128,1024]
    (1 bank) -> ACT/DVE copies into dfT [128h, 8hc, 1024s] stripes;
    dots matmuls lag one stripe behind so copies hide under transposes
  - dots^T = le_n^T @ df^T via 16 accumulating bf16 matmuls; PSUM->SBUF
    scatter-copy makes dotsT_sb column k hold s=k
  - dots^T rows -> natural [128,8] layout via 3 partition-remap DMAs
  - epilogue (rinv scale + mask bias) on DVE; pooled = reduce_max
  - softmax without max-subtraction (|atten| <= 1 + nonpositive bias);
    rinv and 1/Z via exp/ln only (single ACT table set, no reloads)
  - feature_attention via 16 accumulating bf16 matmuls (exp stationary)
  - 2-deep cross-example software pipeline: PE runs example b transposes,
    then b-1 dots-to-natural, then b-2 weighted sum
"""

import sys

sys.path.insert(0, "/opt/trn_rl_repo")

import numpy as np

import concourse.bass as bass
from concourse import mybir
from concourse.bass_types import AP
from concourse.bass_utils import run_bass_kernel_spmd
from concourse.masks import make_identity
from concourse.tile import TileContext

F32 = mybir.dt.float32
BF16 = mybir.dt.bfloat16
AF = mybir.ActivationFunctionType
ALU = mybir.AluOpType
AX = mybir.AxisListType

N_CORES = 8
B, S, H, L = 64, 1024, 1024, 3
B_LOC = B // N_CORES  # 8 examples per core
P = 128
NC = S // P  # 8 chunks (same count for S and H)
EPS = 1e-12

_CACHE = {}

# --- workaround: this walrus build allows at most ONE sync wait per
# instruction ("Too many sync wait commands").  Tile attaches several.
# Split extra waits onto standalone NoOps (same engine, just before the
# instruction) at BIR-serialization time.
_ORIG_TO_JSON_BYTES = bass.Bass.to_json_bytes


def _to_json_bytes_split_waits(self):
    import orjson

    j = orjson.loads(_ORIG_TO_JSON_BYTES(self))
    n = 0
    for fn in j["functions"]:
        for blk in fn["blocks"]:
            insts = blk["instructions"]
            if not any(
                len((i.get("sync_info") or {}).get("on_wait") or []) > 1
                for i in insts
            ):
                continue
            out = []
            for inst in insts:
                si = inst.get("sync_info")
                waits = (si or {}).get("on_wait") or []
                if len(waits) > 1:
                    for w in waits[:-1]:
                        n += 1
                        out.append(
                            {
                                "debug": inst.get("debug", 0),
                                "engine": inst["engine"],
                                "ins": [],
                                "outs": [],
                                "name": f"WSPLIT-{n}",
                                "opcode": "NoOp",
                                "sync_info": {"on_update": [], "on_wait": [w]},
                            }
                        )
                    si["on_wait"] = [waits[-1]]
                out.append(inst)
            blk["instructions"] = out
    return orjson.dumps(j)


bass.Bass.to_json_bytes = _to_json_bytes_split_waits


def _bcast3(ap2: AP, n: int) -> AP:
    """[P, C] AP -> [P, C, n] with a 0-step broadcast innermost dim."""
    return AP(tensor=ap2.tensor, offset=ap2.offset, ap=[*ap2.ap, [0, n]])


def _build():
    nc = bass.Bass("TRN2", target_bir_lowering=False, debug=False)

    df = nc.dram_tensor("deep_features", [B_LOC, S, H], F32, kind="ExternalInput").ap()
    le = nc.dram_tensor("label_embeddings", [H, L], F32, kind="ExternalInput").ap()
    mk = nc.dram_tensor("attention_mask", [B_LOC, S], F32, kind="ExternalInput").ap()
    fa = nc.dram_tensor("feature_attention", [B_LOC, H], F32, kind="ExternalOutput").ap()
    at = nc.dram_tensor("atten", [B_LOC, S, L], F32, kind="ExternalOutput").ap()
    nm = nc.dram_tensor("normalized", [B_LOC, 1, S], F32, kind="ExternalOutput").ap()

    # DRAM views
    # s index mapping: s = p*NC + c  (contiguous 32KB per partition in DRAM)
    df_v = df.rearrange("b (p c) h -> b p c h", c=NC)       # [B,128,8,1024]
    at_v = at.rearrange("b (p c) l -> b p c l", c=NC)       # [B,128,8,3]
    mk_v = mk.rearrange("b (p c) -> b p c", c=NC)           # [B,128,8]
    nm_v = nm.rearrange("b one (p c) -> b p (one c)", c=NC) # [B,128,8]
    le_v = le.rearrange("(c p) l -> p c l", p=P)            # [128,8,3]

    with TileContext(nc) as tc:
        with (
            tc.tile_pool(name="consts", bufs=1) as consts,
            tc.tile_pool(name="dfp", bufs=4) as dfp,
            tc.tile_pool(name="dftp", bufs=2) as dftp,
            tc.tile_pool(name="scr", bufs=2) as scr,
            tc.tile_pool(name="small", bufs=3) as small,
            tc.tile_pool(name="outs", bufs=3) as outs,
            tc.tile_pool(name="ps_tr", bufs=3, space="PSUM") as ps_tr,
            tc.tile_pool(name="ps_dots", bufs=1, space="PSUM") as ps_dots,
            tc.tile_pool(name="ps_fa", bufs=1, space="PSUM") as ps_fa,
        ):
            # ---- constants ----
            ident = consts.tile([P, P], F32)
            make_identity(nc, ident)
            identb = consts.tile([P, P], BF16)
            make_identity(nc, identb)
            ones_big = consts.tile([P, P], F32)
            nc.vector.memset(ones_big, 1.0)

            # ---- normalize label embeddings once ----
            le_sb = consts.tile([P, NC, L], F32)
            nc.sync.dma_start(out=le_sb, in_=le_v)
            le_sq = consts.tile([P, NC, L], F32)
            nc.scalar.activation(le_sq, le_sb, AF.Square)
            # per-partition partial sums over c: view [p, l, c], reduce X
            le_sq_plc = AP(
                tensor=le_sq.tensor,
                offset=le_sq.offset,
                ap=[le_sq.ap[0], [1, L], [L, NC]],
            )
            le_part = consts.tile([P, L], F32)
            nc.vector.tensor_reduce(le_part, le_sq_plc, axis=AX.X, op=ALU.add)
            lep_tile = ps_fa.tile([P, L], F32, tag="fa")
            lep_ps = lep_tile[:, :]
            # all-ones stationary: every out partition gets the col sum
            nc.tensor.matmul(lep_ps, ones_big, le_part)  # [128,L] col sums
            # 1/max(sqrt(x), eps) == exp(-0.5*ln(max(x, eps^2))); stays in
            # the natural_log_exp ACT table set (no per-example set switch)
            le_lns = consts.tile([P, L], F32)
            nc.vector.tensor_scalar_max(le_lns, lep_ps, EPS * EPS)
            nc.scalar.activation(le_lns, le_lns, AF.Ln)
            le_rinv_bc = consts.tile([P, L], F32)
            nc.scalar.activation(le_rinv_bc, le_lns, AF.Exp, scale=-0.5)
            le_n = consts.tile([P, NC, L], F32)
            for c in range(NC):
                nc.vector.tensor_tensor(
                    out=le_n[:, c, :], in0=le_sb[:, c, :], in1=le_rinv_bc, op=ALU.mult
                )
            le_nb = consts.tile([P, NC, L], BF16)
            nc.vector.tensor_copy(le_nb, le_n)

            # ---- per-example pipeline (2-deep software pipeline) ----
            # PE emission order per step: [transposes+dots of b] then
            # [minis of b-1] then [wsum of b-2], so PE never waits on the
            # serial softmax chain of the current example.
            st = {}

            def phase_load(b):
                s = {}
                df_t = dfp.tile([P, NC, H], BF16, tag="df")
                nc.gpsimd.dma_start(out=df_t[:, 0:4, :], in_=df_v[b][:, 0:4, :])
                nc.gpsimd.dma_start(out=df_t[:, 4:8, :], in_=df_v[b][:, 4:8, :])
                s["df_t"] = df_t

                ss = small.tile([P, NC], F32, tag="ss")
                for c in range(NC):
                    sq = scr.tile([P, H], BF16, tag="sq")
                    if c < 4:
                        nc.scalar.activation(
                            sq, df_t[:, c, :], AF.Square, accum_out=ss[:, c : c + 1]
                        )
                    else:
                        nc.vector.scalar_tensor_tensor(
                            out=sq,
                            in0=df_t[:, c, :],
                            scalar=1.0,
                            in1=df_t[:, c, :],
                            op0=ALU.mult,
                            op1=ALU.mult,
                            accum_out=ss[:, c : c + 1],
                        )
                # rinv = 1/max(sqrt(ss), eps) = exp(-0.5*ln(max(ss, eps^2)))
                rt = small.tile([P, NC], F32, tag="rt")
                nc.vector.tensor_scalar_max(rt, ss, EPS * EPS)
                nc.scalar.activation(rt, rt, AF.Ln)
                rinv = small.tile([P, NC], F32, tag="rinv")
                nc.scalar.activation(rinv, rt, AF.Exp, scale=-0.5)
                s["rinv"] = rinv

                mask_nat = small.tile([P, NC], F32, tag="mask")
                nc.sync.dma_start(out=mask_nat, in_=mk_v[b])
                bias_nat = small.tile([P, NC], F32, tag="bias")
                nc.scalar.activation(
                    bias_nat, mask_nat, AF.Copy, bias=-10000.0, scale=10000.0
                )
                s["bias_nat"] = bias_nat
                return s

            def phase_transpose_dots(b, s):
                df_t = s["df_t"]
                dfT = dftp.tile([P, NC, S], BF16, tag="dfT")
                dots0 = ps_dots.tile([L, 512], F32, tag="dots0")
                dots1 = ps_dots.tile([L, 512], F32, tag="dots1")

                def dots_mm(hc):
                    lhsT = le_nb[:, hc, :]
                    nc.tensor.matmul(
                        dots0,
                        lhsT,
                        dfT[:, hc, 0:512],
                        start=(hc == 0),
                        stop=(hc == NC - 1),
                        skip_group_check=True,
                    )
                    nc.tensor.matmul(
                        dots1,
                        lhsT,
                        dfT[:, hc, 512:1024],
                        start=(hc == 0),
                        stop=(hc == NC - 1),
                        skip_group_check=True,
                    )

                # dots for hc are emitted one step behind the transposes so
                # the PSUM->SBUF copy of stripe hc hides under the hc+1
                # transpose burst (PE never waits on the copy)
                for hc in range(NC):
                    tr = ps_tr.tile([P, S], BF16, tag="tr")  # 2KB = 1 bank
                    for sc in range(NC):
                        nc.tensor.transpose(
                            tr[:, sc * P : (sc + 1) * P],
                            df_t[:, sc, hc * P : (hc + 1) * P],
                            identb,
                        )
                    dst = dfT[:, hc, :]
                    if hc % 2 == 0:
                        nc.scalar.copy(dst, tr)
                    else:
                        nc.vector.tensor_copy(dst, tr)
                    if hc > 0:
                        dots_mm(hc - 1)
                dots_mm(NC - 1)

                dotsT_sb = small.tile([L, S], F32, tag="dotsT")
                # scatter copy so dotsT_sb column k holds s=k (undo the
                # block ordering of the transposed stripes)
                dT_v = dotsT_sb.rearrange("l (j c) -> l c j", c=NC)
                nc.vector.tensor_copy(
                    dT_v[:, 0:4, :], dots0.rearrange("l (c j) -> l c j", c=4)
                )
                nc.vector.tensor_copy(
                    dT_v[:, 4:8, :], dots1.rearrange("l (c j) -> l c j", c=4)
                )
                s["dotsT_sb"] = dotsT_sb
            def phase_softmax(b, s):
                dotsT_sb = s["dotsT_sb"]
                # dots^T rows -> natural (p c) layout via partition-remap DMAs
                dnat_sb = small.tile([P, L, NC], F32, tag="dnat")
                for l in range(L):
                    nc.sync.dma_start(
                        out=dnat_sb[:, l, :], in_=dotsT_sb[l : l + 1, :]
                    )
                tmp3 = outs.tile([P, NC, L], F32, tag="tmp3")
                atnat = outs.tile([P, NC, L], F32, tag="atnat")
                for l in range(L):
                    nc.vector.tensor_tensor(
                        out=tmp3[:, :, l],
                        in0=dnat_sb[:, l, :],
                        in1=s["rinv"],
                        op=ALU.mult,
                    )
                    nc.vector.tensor_tensor(
                        out=atnat[:, :, l],
                        in0=tmp3[:, :, l],
                        in1=s["bias_nat"],
                        op=ALU.add,
                    )
                nc.scalar.dma_start(out=at_v[b], in_=atnat)
                pooled = small.tile([P, NC], F32, tag="pooled")
                nc.vector.tensor_reduce(pooled, atnat, axis=AX.X, op=ALU.max)

                exp_nat = small.tile([P, NC], F32, tag="exp")
                esum_col = small.tile([P, 1], F32, tag="esumc")
                nc.scalar.activation(exp_nat, pooled, AF.Exp, accum_out=esum_col)
                esum_tile = ps_dots.tile([P, 1], F32, tag="esum")
                esum_ps = esum_tile[:, :]
                nc.tensor.matmul(esum_ps, ones_big, esum_col)
                rz128 = small.tile([P, 1], F32, tag="rz128")
                nc.vector.reciprocal(rz128, esum_ps)
                s["rz128"] = rz128

                norm_nat = outs.tile([P, NC], F32, tag="norm")
                nc.vector.tensor_scalar_mul(norm_nat, exp_nat, rz128)
                nc.gpsimd.dma_start(out=nm_v[b], in_=norm_nat)

                exp_bf = small.tile([P, NC], BF16, tag="expbf")
                nc.vector.tensor_copy(exp_bf, exp_nat)
                s["exp_bf"] = exp_bf

            def phase_wsum(b, s):
                df_t, exp_bf, rz128 = s["df_t"], s["exp_bf"], s["rz128"]
                fa_ps = ps_fa.tile([1, H], F32, tag="fa")
                for c in range(NC):
                    lhsT = exp_bf[:, c : c + 1]
                    nc.tensor.matmul(
                        fa_ps[:, 0:512],
                        lhsT,
                        df_t[:, c, 0:512],
                        start=(c == 0),
                        stop=(c == NC - 1),
                        skip_group_check=True,
                    )
                    nc.tensor.matmul(
                        fa_ps[:, 512:1024],
                        lhsT,
                        df_t[:, c, 512:1024],
                        start=(c == 0),
                        stop=(c == NC - 1),
                        skip_group_check=True,
                    )
                fa_sb = outs.tile([1, H], F32, tag="fa_sb")
                nc.vector.tensor_scalar_mul(
                    fa_sb[:, 0:512], fa_ps[:, 0:512], rz128[0:1, :]
                )
                nc.vector.tensor_scalar_mul(
                    fa_sb[:, 512:1024], fa_ps[:, 512:1024], rz128[0:1, :]
                )
                nc.gpsimd.dma_start(out=fa[b : b + 1, :], in_=fa_sb)

            for b in range(B_LOC + 2):
                if b < B_LOC:
                    st[b] = phase_load(b)
                    phase_transpose_dots(b, st[b])
                if 1 <= b <= B_LOC:
                    phase_softmax(b - 1, st[b - 1])
                if b >= 2:
                    phase_wsum(b - 2, st[b - 2])
                    del st[b - 2]

    return nc


def get_nc():
    if "nc" not in _CACHE:
        _CACHE["nc"] = _build()
    return _CACHE["nc"]


def kernel(deep_features, label_embeddings, attention_mask, trace=False, **kw):
    deep_features = np.asarray(deep_features, dtype=np.float32)
    label_embeddings = np.asarray(label_embeddings, dtype=np.float32)
    attention_mask = np.asarray(attention_mask, dtype=np.float32)

    nc = get_nc()
    in_maps = []
    for i in range(N_CORES):
        sl = slice(i * B_LOC, (i + 1) * B_LOC)
        in_maps.append(
            {
                "deep_features": np.ascontiguousarray(deep_features[sl]),
                "label_embeddings": label_embeddings,
                "attention_mask": np.ascontiguousarray(attention_mask[sl]),
            }
        )
    res = run_bass_kernel_spmd(
        nc, in_maps, core_ids=list(range(N_CORES)), trace=trace, **kw
    )
    fa = np.concatenate([r["feature_attention"] for r in res.results], axis=0)
    at = np.concatenate([r["atten"] for r in res.results], axis=0)
    nm = np.concatenate([r["normalized"] for r in res.results], axis=0)
    if trace:
        kernel.last_results = res
    return fa, at, nm


# revision 38
# speedup vs baseline: 1.1219x; 1.0645x over previous
"""AttentionLayer Bass kernel for 8 trn2 NeuronCores.

Math (per example b):
    rinv[s]  = 1/max(||df[b,s,:]||, eps)
    le_n     = le / max(||le[:,l]||, eps)          (per column l)
    dots     = df @ le_n                            [S,3]
    atten    = dots * rinv[:,None] + 10000*(mask-1)[:,None]
    pooled   = max_l atten                          [S]
    normalized = softmax_s(pooled)
    feature_attention = normalized @ df             [H]

Sharding: data-parallel over batch B=64 -> 8 examples per core.

Layout strategy per example (s index mapping: s = p*8 + c):
  - df loaded as bf16 [128p, 8c, 1024h] via two SWDGE cast-DMAs (fp32->bf16)
  - row sumsq: ACT Square+accum (4 chunks) + DVE scalar_tensor_tensor (4)
  - df^T built with 64 PE transpose-mode matmuls (bf16) -> PSUM [128,1024]
    (1 bank) -> ACT/DVE copies into dfT [128h, 8hc, 1024s] stripes;
    dots matmuls lag one stripe behind so copies hide under transposes
  - dots^T = le_n^T @ df^T via 16 accumulating bf16 matmuls; PSUM->SBUF
    scatter-copy makes dotsT_sb column k hold s=k
  - dots^T rows -> natural [128,8] layout via 3 partition-remap DMAs
  - epilogue (rinv scale + mask bias) on DVE; pooled = reduce_max
  - softmax without max-subtraction (|atten| <= 1 + nonpositive bias);
    rinv and 1/Z via exp/ln only (single ACT table set, no reloads)
  - feature_attention via 16 accumulating bf16 matmuls (exp stationary)
  - 2-deep cross-example software pipeline: PE runs example b transposes,
    then b-1 dots-to-natural, then b-2 weighted sum
"""

import sys

sys.path.insert(0, "/opt/trn_rl_repo")

import numpy as np

import concourse.bass as bass
from concourse import mybir
from concourse.bass_types import AP
from concourse.bass_utils import run_bass_kernel_spmd
from concourse.masks import make_identity
from concourse.tile import TileContext

F32 = mybir.dt.float32
BF16 = mybir.dt.bfloat16
AF = mybir.ActivationFunctionType
ALU = mybir.AluOpType
AX = mybir.AxisListType

N_CORES = 8
B, S, H, L = 64, 1024, 1024, 3
B_LOC = B // N_CORES  # 8 examples per core
P = 128
NC = S // P  # 8 chunks (same count for S and H)
EPS = 1e-12

_CACHE = {}

# --- workaround: this walrus build allows at most ONE sync wait per
# instruction ("Too many sync wait commands").  Tile attaches several.
# Split extra waits onto standalone NoOps (same engine, just before the
# instruction) at BIR-serialization time.
_ORIG_TO_JSON_BYTES = bass.Bass.to_json_bytes


def _to_json_bytes_split_waits(self):
    import orjson

    j = orjson.loads(_ORIG_TO_JSON_BYTES(self))
    n = 0
    for fn in j["functions"]:
        for blk in fn["blocks"]:
            insts = blk["instructions"]
            if not any(
                len((i.get("sync_info") or {}).get("on_wait") or []) > 1
                for i in insts
            ):
                continue
            out = []
            for inst in insts:
                si = inst.get("sync_info")
                waits = (si or {}).get("on_wait") or []
                if len(waits) > 1:
                    for w in waits[:-1]:
                        n += 1
                        out.append(
                            {
                                "debug": inst.get("debug", 0),
                                "engine": inst["engine"],
                                "ins": [],
                                "outs": [],
                                "name": f"WSPLIT-{n}",
                                "opcode": "NoOp",
                                "sync_info": {"on_update": [], "on_wait": [w]},
                            }
                        )
                    si["on_wait"] = [waits[-1]]
                out.append(inst)
            blk["instructions"] = out
    return orjson.dumps(j)


bass.Bass.to_json_bytes = _to_json_bytes_split_waits


def _build():
    nc = bass.Bass("TRN2", target_bir_lowering=False, debug=False)

    df = nc.dram_tensor("deep_features", [B_LOC, S, H], F32, kind="ExternalInput").ap()
    le = nc.dram_tensor("label_embeddings", [H, L], F32, kind="ExternalInput").ap()
    mk = nc.dram_tensor("attention_mask", [B_LOC, S], F32, kind="ExternalInput").ap()
    fa = nc.dram_tensor("feature_attention", [B_LOC, H], F32, kind="ExternalOutput").ap()
    at = nc.dram_tensor("atten", [B_LOC, S, L], F32, kind="ExternalOutput").ap()
    nm = nc.dram_tensor("normalized", [B_LOC, 1, S], F32, kind="ExternalOutput").ap()

    # DRAM views
    # s index mapping: s = p*NC + c  (contiguous 32KB per partition in DRAM)
    df_v = df.rearrange("b (p c) h -> b p c h", c=NC)       # [B,128,8,1024]
    at_v = at.rearrange("b (p c) l -> b p c l", c=NC)       # [B,128,8,3]
    mk_v = mk.rearrange("b (p c) -> b p c", c=NC)           # [B,128,8]
    nm_v = nm.rearrange("b one (p c) -> b p (one c)", c=NC) # [B,128,8]
    le_v = le.rearrange("(c p) l -> p c l", p=P)            # [128,8,3]

    with TileContext(nc) as tc:
        with (
            tc.tile_pool(name="consts", bufs=1) as consts,
            tc.tile_pool(name="dfp", bufs=4) as dfp,
            tc.tile_pool(name="dftp", bufs=2) as dftp,
            tc.tile_pool(name="scr", bufs=2) as scr,
            tc.tile_pool(name="small", bufs=3) as small,
            tc.tile_pool(name="outs", bufs=3) as outs,
            tc.tile_pool(name="ps_tr", bufs=3, space="PSUM") as ps_tr,
            tc.tile_pool(name="ps_dots", bufs=1, space="PSUM") as ps_dots,
            tc.tile_pool(name="ps_fa", bufs=1, space="PSUM") as ps_fa,
        ):
            # ---- constants ----
            identb = consts.tile([P, P], BF16)
            make_identity(nc, identb)
            ones_big = consts.tile([P, P], F32)
            nc.vector.memset(ones_big, 1.0)

            # ---- normalize label embeddings once ----
            le_sb = consts.tile([P, NC, L], F32)
            nc.sync.dma_start(out=le_sb, in_=le_v)
            le_sq = consts.tile([P, NC, L], F32)
            nc.scalar.activation(le_sq, le_sb, AF.Square)
            # per-partition partial sums over c: view [p, l, c], reduce X
            le_sq_plc = AP(
                tensor=le_sq.tensor,
                offset=le_sq.offset,
                ap=[le_sq.ap[0], [1, L], [L, NC]],
            )
            le_part = consts.tile([P, L], F32)
            nc.vector.tensor_reduce(le_part, le_sq_plc, axis=AX.X, op=ALU.add)
            lep_tile = ps_fa.tile([P, L], F32, tag="fa")
            lep_ps = lep_tile[:, :]
            # all-ones stationary: every out partition gets the col sum
            nc.tensor.matmul(lep_ps, ones_big, le_part)  # [128,L] col sums
            # 1/max(sqrt(x), eps) == exp(-0.5*ln(max(x, eps^2))); stays in
            # the natural_log_exp ACT table set (no per-example set switch)
            le_lns = consts.tile([P, L], F32)
            nc.vector.tensor_scalar_max(le_lns, lep_ps, EPS * EPS)
            nc.scalar.activation(le_lns, le_lns, AF.Ln)
            le_rinv_bc = consts.tile([P, L], F32)
            nc.scalar.activation(le_rinv_bc, le_lns, AF.Exp, scale=-0.5)
            le_n = consts.tile([P, NC, L], F32)
            for c in range(NC):
                nc.vector.tensor_tensor(
                    out=le_n[:, c, :], in0=le_sb[:, c, :], in1=le_rinv_bc, op=ALU.mult
                )
            le_nb = consts.tile([P, NC, L], BF16)
            nc.vector.tensor_copy(le_nb, le_n)

            # ---- per-example pipeline (2-deep software pipeline) ----
            # PE emission order per step: [transposes+dots of b] then
            # [minis of b-1] then [wsum of b-2], so PE never waits on the
            # serial softmax chain of the current example.
            st = {}

            def phase_load(b):
                s = {}
                df_t = dfp.tile([P, NC, H], BF16, tag="df")
                nc.gpsimd.dma_start(out=df_t[:, 0:4, :], in_=df_v[b][:, 0:4, :])
                nc.gpsimd.dma_start(out=df_t[:, 4:8, :], in_=df_v[b][:, 4:8, :])
                s["df_t"] = df_t

                ss = small.tile([P, NC], F32, tag="ss")
                for c in range(NC):
                    sq = scr.tile([P, H], BF16, tag="sq")
                    if c < 4:
                        nc.scalar.activation(
                            sq, df_t[:, c, :], AF.Square, accum_out=ss[:, c : c + 1]
                        )
                    else:
                        nc.vector.scalar_tensor_tensor(
                            out=sq,
                            in0=df_t[:, c, :],
                            scalar=1.0,
                            in1=df_t[:, c, :],
                            op0=ALU.mult,
                            op1=ALU.mult,
                            accum_out=ss[:, c : c + 1],
                        )
                # rinv = 1/max(sqrt(ss), eps) = exp(-0.5*ln(max(ss, eps^2)))
                rt = small.tile([P, NC], F32, tag="rt")
                nc.vector.tensor_scalar_max(rt, ss, EPS * EPS)
                nc.scalar.activation(rt, rt, AF.Ln)
                rinv = small.tile([P, NC], F32, tag="rinv")
                nc.scalar.activation(rinv, rt, AF.Exp, scale=-0.5)
                s["rinv"] = rinv

                mask_nat = small.tile([P, NC], F32, tag="mask")
                nc.sync.dma_start(out=mask_nat, in_=mk_v[b])
                bias_nat = small.tile([P, NC], F32, tag="bias")
                nc.scalar.activation(
                    bias_nat, mask_nat, AF.Copy, bias=-10000.0, scale=10000.0
                )
                s["bias_nat"] = bias_nat
                return s

            def phase_transpose_dots(b, s):
                df_t = s["df_t"]
                dfT = dftp.tile([P, NC, S], BF16, tag="dfT")
                dots0 = ps_dots.tile([L, 512], F32, tag="dots0")
                dots1 = ps_dots.tile([L, 512], F32, tag="dots1")

                def dots_mm(hc):
                    lhsT = le_nb[:, hc, :]
                    nc.tensor.matmul(
                        dots0,
                        lhsT,
                        dfT[:, hc, 0:512],
                        start=(hc == 0),
                        stop=(hc == NC - 1),
                        skip_group_check=True,
                    )
                    nc.tensor.matmul(
                        dots1,
                        lhsT,
                        dfT[:, hc, 512:1024],
                        start=(hc == 0),
                        stop=(hc == NC - 1),
                        skip_group_check=True,
                    )

                # dots for hc are emitted one step behind the transposes so
                # the PSUM->SBUF copy of stripe hc hides under the hc+1
                # transpose burst (PE never waits on the copy)
                for hc in range(NC):
                    tr = ps_tr.tile([P, S], BF16, tag="tr")  # 2KB = 1 bank
                    for sc in range(NC):
                        nc.tensor.transpose(
                            tr[:, sc * P : (sc + 1) * P],
                            df_t[:, sc, hc * P : (hc + 1) * P],
                            identb,
                        )
                    dst = dfT[:, hc, :]
                    if hc % 2 == 0:
                        nc.scalar.copy(dst, tr)
                    else:
                        nc.vector.tensor_copy(dst, tr)
                    if hc > 0:
                        dots_mm(hc - 1)
                dots_mm(NC - 1)

                dotsT_sb = small.tile([L, S], F32, tag="dotsT")
                # scatter copy so dotsT_sb column k holds s=k (undo the
                # block ordering of the transposed stripes)
                dT_v = dotsT_sb.rearrange("l (j c) -> l c j", c=NC)
                nc.vector.tensor_copy(
                    dT_v[:, 0:4, :], dots0.rearrange("l (c j) -> l c j", c=4)
                )
                nc.vector.tensor_copy(
                    dT_v[:, 4:8, :], dots1.rearrange("l (c j) -> l c j", c=4)
                )
                s["dotsT_sb"] = dotsT_sb
            def phase_softmax(b, s):
                dotsT_sb = s["dotsT_sb"]
                # dots^T rows -> natural (p c) layout via partition-remap DMAs
                dnat_sb = small.tile([P, L, NC], F32, tag="dnat")
                for l in range(L):
                    nc.sync.dma_start(
                        out=dnat_sb[:, l, :], in_=dotsT_sb[l : l + 1, :]
                    )
                tmp3 = outs.tile([P, NC, L], F32, tag="tmp3")
                atnat = outs.tile([P, NC, L], F32, tag="atnat")
                for l in range(L):
                    nc.vector.tensor_tensor(
                        out=tmp3[:, :, l],
                        in0=dnat_sb[:, l, :],
                        in1=s["rinv"],
                        op=ALU.mult,
                    )
                    nc.vector.tensor_tensor(
                        out=atnat[:, :, l],
                        in0=tmp3[:, :, l],
                        in1=s["bias_nat"],
                        op=ALU.add,
                    )
                nc.scalar.dma_start(out=at_v[b], in_=atnat)
                pooled = small.tile([P, NC], F32, tag="pooled")
                nc.vector.tensor_reduce(pooled, atnat, axis=AX.X, op=ALU.max)

                exp_nat = small.tile([P, NC], F32, tag="exp")
                esum_col = small.tile([P, 1], F32, tag="esumc")
                nc.scalar.activation(exp_nat, pooled, AF.Exp, accum_out=esum_col)
                esum_tile = ps_dots.tile([P, 1], F32, tag="esum")
                esum_ps = esum_tile[:, :]
                nc.tensor.matmul(esum_ps, ones_big, esum_col)
                rz128 = small.tile([P, 1], F32, tag="rz128")
                nc.vector.reciprocal(rz128, esum_ps)
                s["rz128"] = rz128

                norm_nat = outs.tile([P, NC], F32, tag="norm")
                nc.vector.tensor_scalar_mul(norm_nat, exp_nat, rz128)
                nc.gpsimd.dma_start(out=nm_v[b], in_=norm_nat)

                exp_bf = small.tile([P, NC], BF16, tag="expbf")
                nc.vector.tensor_copy(exp_bf, exp_nat)
                s["exp_bf"] = exp_bf

            def phase_wsum(b, s):
                df_t, exp_bf, rz128 = s["df_t"], s["exp_bf"], s["rz128"]
                fa_ps = ps_fa.tile([1, H], F32, tag="fa")
                for c in range(NC):
                    lhsT = exp_bf[:, c : c + 1]
                    nc.tensor.matmul(
                        fa_ps[:, 0:512],
                        lhsT,
                        df_t[:, c, 0:512],
                        start=(c == 0),
                        stop=(c == NC - 1),
                        skip_group_check=True,
                    )
                    nc.tensor.matmul(
                        fa_ps[:, 512:1024],
                        lhsT,
                        df_t[:, c, 512:1024],
                        start=(c == 0),
                        stop=(c == NC - 1),
                        skip_group_check=True,
                    )
                fa_sb = outs.tile([1, H], F32, tag="fa_sb")
                nc.vector.tensor_scalar_mul(
                    fa_sb[:, 0:512], fa_ps[:, 0:512], rz128[0:1, :]
                )
                nc.vector.tensor_scalar_mul(
                    fa_sb[:, 512:1024], fa_ps[:, 512:1024], rz128[0:1, :]
                )
                nc.gpsimd.dma_start(out=fa[b : b + 1, :], in_=fa_sb)

            for b in range(B_LOC + 2):
                if b < B_LOC:
                    st[b] = phase_load(b)
                    phase_transpose_dots(b, st[b])
                if 1 <= b <= B_LOC:
                    phase_softmax(b - 1, st[b - 1])
                if b >= 2:
                    phase_wsum(b - 2, st[b - 2])
                    del st[b - 2]

    return nc


def get_nc():
    if "nc" not in _CACHE:
        _CACHE["nc"] = _build()
    return _CACHE["nc"]


def kernel(deep_features, label_embeddings, attention_mask, trace=False, **kw):
    deep_features = np.asarray(deep_features, dtype=np.float32)
    label_embeddings = np.asarray(label_embeddings, dtype=np.float32)
    attention_mask = np.asarray(attention_mask, dtype=np.float32)

    nc = get_nc()
    in_maps = []
    for i in range(N_CORES):
        sl = slice(i * B_LOC, (i + 1) * B_LOC)
        in_maps.append(
            {
                "deep_features": np.ascontiguousarray(deep_features[sl]),
                "label_embeddings": label_embeddings,
                "attention_mask": np.ascontiguousarray(attention_mask[sl]),
            }
        )
    res = run_bass_kernel_spmd(
        nc, in_maps, core_ids=list(range(N_CORES)), trace=trace, **kw
    )
    fa = np.concatenate([r["feature_attention"] for r in res.results], axis=0)
    at = np.concatenate([r["atten"] for r in res.results], axis=0)
    nm = np.concatenate([r["normalized"] for r in res.results], axis=0)
    if trace:
        kernel.last_results = res
    return fa, at, nm
